# revision 1
# baseline (speedup 1.0000x reference)
"""Causal attention (B=8, N=4096, D=64) on 8 trn2 NeuronCores.

Sharding: batch b -> core b (data parallel, no cross-core comms).

Per-core kernel (flash-attention style, fully transposed dataflow -- no
on-chip transposes anywhere):
  inputs (host pre-layouts, fp16):
    kpair [128, 16, 128]  kT tile pairs: partitions 0-63 = kT of even
                          tiles, 64-127 = kT of odd tiles (d on partitions)
    qq    [128, 8, 512]   qT blocks, duplicated on both partition halves
    v_aug [128, 32, 65]   k-tiled; col 64 = 1.0; padding-masked rows = 0
    tri   [128, 128]      lower-triangular 0/1 (keep where qcol >= krow)
  for each q-block (512 wide, ascending), each causal k-tile PAIR:
    MM1 x2 ROW-TILED: logitsT[k,q] = matmul(kT_t [64,128], qT [64,512])
      The two K=64 matmuls of a pair sit at partition bases 0 and 64 ->
      tile_position (0,0)/(64,0) -> they run CONCURRENTLY in different
      row strips of the PE array (~518 cyc/pair instead of 1024).
    expT = exp(logitsT_pair / sqrt(d))  one ACT op over [128,1024] -> SBUF
    diagonal tiles: pb strip [128,128] *= tri                       (DVE)
    MM2 x2: outT[d,q] (+)= matmul(v_aug [128,65], expT [128,cs:512]) PSUM
      -- v_aug col 64 is 1.0 => outT row 64 = the softmax denominators
      -- diagonal tiles only touch their live columns [128j, 512): saves
         ~6k PE cycles/block and keeps the tri mask to one 128-wide strip
  The far-diagonal pair (tiles j=2,3) exps only its live strided slice
  [*, :, 256:] -- one ACT op at half the cost.
  The MM2s are emitted one pair behind the MM1s/exp so the PE stream is
  [.. MM1s(p) MM2s(p-1) ..] and the exp latency is hidden.
  per q-block epilogue (all off the PE, DMAs on the idle GpSimd queue):
  copy sums row + numerators PSUM->SBUF immediately (releases the acc
  PSUM bank early -- no MM2 stall at block qb+2, and custom-DVE ops
  misread PSUM operands on HW); reciprocal_approx_fast; DMA 1/s to DRAM
  and back broadcast across 64 partitions (partition-step-0 reads are
  DRAM-only); out = nums * rb on DVE; DMA to outT_dram[:, q-block].
  The LAST q-block ships raw numerators + sums instead (nothing left to
  hide the ~5us DMA round-trip behind); the host gather performs that
  one block's divide. Host transposes outT_dram [64, N] -> [N, 64].

Padding mask: host zeroes masked k rows of v_aug (incl. the ones column),
so masked keys contribute nothing to numerator or denominator -- exactly
equivalent to -inf logits.

Matmul operands are fp16 (1 cycle/row on the PE; fp32 PSUM accumulation);
measured rel err vs the fp32 reference is ~4e-4.

Perf notes (measured on trn2, 8 cores SPMD): 90.8-92.8us/core on a
rested device (best 90.76us; ~108-110 when back-to-back benching leaves
the ACT engine pre-throttled -- a ~7min idle restores it), vs
~120-159us for the previous kernel. The input-DMA order below is the
measured optimum of five schedules: qq[1]/qq[2] ride ahead of the v
chunk so blocks 1-2 never stall the exp stream (trace-verified
block-boundary deltas 570/889ns vs the 997ns ACT pace); hoisting
kpair[2:4] as well was tried and made the stream worse. The steady state is ACT-bound (the
exp stream saturates the Scalar engine; PE/DVE/DMA all have slack), so
q-block order is ascending: small blocks warm the HAM clock-gate early
(PE 1.2->2.4GHz after ~3.4us of continuous matmuls) and the one exposed
epilogue lands after the largest block. ACT exp throughput is the
architectural floor here: ~0.833ns/logit, the same rate as the PE's
2 matmul-cycles/logit at full clock. Sustained 100% ACT duty also
draws a slow progressive engine throttle (~+7ns/us on exp duration),
which shorter kernels accumulate less of. Dependency-free warmup ops
on the Scalar/Vector queues hide their first-use setup (TENSOR_LOAD,
ACT_TABLE_LOAD, ~3.5us) inside the fixed ~7us framework preamble.
"""

import os
from contextlib import ExitStack

import numpy as np

B, N, D = 8, 4096, 64
QBLK = 512
KTILE = 128

LAST_RESULTS = None
_NC_CACHE = {}


def build(n=N, d=D, qblk=QBLK, ktile=KTILE, lg_bufs=3, acc_bufs=2, pb_bufs=6,
          mm2_defer=1):
    import concourse.bass as bass
    import concourse.mybir as mybir
    import concourse.tile as tile
    from concourse import bacc

    f32 = mybir.dt.float32
    f16 = mybir.dt.float16
    nt = n // ktile          # number of k-tiles (32)
    nqb = n // qblk          # number of q-blocks (8)
    tpq = qblk // ktile      # k-tiles per q-block (diagonal span, 4)
    npr = nt // 2            # k-tile pairs total (16)

    nc = bacc.Bacc("TRN2", target_bir_lowering=False, debug=False,
                   enable_asserts=False)

    kp_d = nc.dram_tensor("kpair", (2 * d, npr, ktile), f16,
                          kind="ExternalInput").ap()
    qq_d = nc.dram_tensor("qq", (2 * d, nqb, qblk), f16,
                          kind="ExternalInput").ap()
    v_d = nc.dram_tensor("v_aug", (ktile, nt, d + 1), f16,
                         kind="ExternalInput").ap()
    tri_d = nc.dram_tensor("tri", (ktile, ktile), f16,
                           kind="ExternalInput").ap()
    oT_d = nc.dram_tensor("outT", (d, n), f32, kind="ExternalOutput").ap()
    rs_d = nc.dram_tensor("rs_scratch", (nqb, qblk), f32,
                          kind="Internal").ap()
    ns_d = nc.dram_tensor("nsum", (1, qblk), f32, kind="ExternalOutput").ap()

    scale = 1.0 / float(np.sqrt(d))

    with tile.TileContext(nc) as tc:
        with ExitStack() as ctx:
            singles = ctx.enter_context(tc.tile_pool(name="singles", bufs=1))
            pb_pool = ctx.enter_context(tc.tile_pool(name="pb", bufs=pb_bufs))
            ob_pool = ctx.enter_context(tc.tile_pool(name="ob", bufs=6))
            lg_pool = ctx.enter_context(
                tc.tile_pool(name="lg", bufs=lg_bufs, space="PSUM"))
            acc_pool = ctx.enter_context(
                tc.tile_pool(name="acc", bufs=acc_bufs, space="PSUM"))

            # --- resident inputs -------------------------------------------
            kp_sb = singles.tile([2 * d, npr, ktile], f16)
            qq_sb = singles.tile([2 * d, nqb, qblk], f16)
            v_sb = singles.tile([ktile, nt, d + 1], f16)
            tri_sb = singles.tile([ktile, ktile], f16)

            # Engine warmup: the Scalar/Vector engines otherwise run their
            # first-use setup (TENSOR_LOAD ~1us, ACT_TABLE_LOAD ~1.3us)
            # AFTER their first dependency wait resolves -- i.e., on the
            # critical path. A tiny dependency-free op on each queue pulls
            # that setup into the DMA dead zone at kernel start.
            warm = ob_pool.tile([1, 4], f32, name="warm")
            nc.gpsimd.memset(warm, 0.0)
            warm_o = ob_pool.tile([1, 4], f32, name="warm_o")
            nc.scalar.activation(warm_o, warm,
                                 mybir.ActivationFunctionType.Exp,
                                 scale=scale)
            nc.vector.tensor_copy(warm_o, warm)

            # Blocks are processed in ASCENDING order; block qb needs k-tile
            # pairs 0..2qb+1, v tiles 0..4qb+3, and its own q block. Small
            # first chunks so compute starts early; per-DMA issue on the
            # sync queue is ~650ns, serial.
            # Critical-path inputs go first, split across engine queues so
            # their descriptor generation runs in parallel.
            nc.sync.dma_start(out=kp_sb[:, 0:2, :], in_=kp_d[:, 0:2, :])
            nc.sync.dma_start(out=qq_sb[:, 0, :], in_=qq_d[:, 0, :])
            nc.sync.dma_start(out=tri_sb, in_=tri_d)
            nc.sync.dma_start(out=qq_sb[:, 1, :], in_=qq_d[:, 1, :])
            nc.sync.dma_start(out=qq_sb[:, 2, :], in_=qq_d[:, 2, :])
            nc.sync.dma_start(out=v_sb[:, 0:4, :], in_=v_d[:, 0:4, :])
            nc.sync.dma_start(out=qq_sb[:, 3, :], in_=qq_d[:, 3, :])
            nc.sync.dma_start(out=kp_sb[:, 2:8, :], in_=kp_d[:, 2:8, :])
            nc.sync.dma_start(out=v_sb[:, 4:16, :], in_=v_d[:, 4:16, :])
            nc.sync.dma_start(out=qq_sb[:, 4:nqb, :], in_=qq_d[:, 4:nqb, :])
            nc.sync.dma_start(out=kp_sb[:, 8:npr, :], in_=kp_d[:, 8:npr, :])
            nc.sync.dma_start(out=v_sb[:, 16:nt, :], in_=v_d[:, 16:nt, :])

            # --- main loop -------------------------------------------------
            def epilogue(acc, qb):
                # normalize: out = acc[0:64] * (1/sums) (sums = row d of acc).
                # Both acc reads are staged to SBUF immediately (custom-DVE
                # ops misread PSUM operands on HW, and the early copy
                # releases the PSUM acc bank ~4us sooner -- no MM2 stall at
                # the next-next block). A DRAM round-trip broadcasts 1/s
                # across the 64 partitions (partition-step-0 reads are
                # DRAM-only); DMAs ride the idle GpSimd queue.
                qs = qb * qblk
                ssum = ob_pool.tile([1, qblk], f32, name="ssum")
                nc.vector.tensor_copy(ssum, acc[d:d + 1, :])
                nums = ob_pool.tile([d, qblk], f32, name="nums")
                nc.vector.tensor_copy(nums, acc[0:d, :])
                rsum = ob_pool.tile([1, qblk], f32, name="rsum")
                nc.vector.reciprocal_approx_fast(out=rsum, in_=ssum)
                nc.gpsimd.dma_start(out=rs_d[qb:qb + 1, :], in_=rsum)
                rb = ob_pool.tile([d, qblk], f32, name="rb")
                rs_slice = rs_d[qb:qb + 1, :]
                brd = bass.AP(tensor=rs_slice.tensor,
                              offset=rs_slice.offset,
                              ap=[[0, d], list(rs_slice.ap[-1])])
                nc.gpsimd.dma_start(out=rb, in_=brd)
                ob = ob_pool.tile([d, qblk], f32, name="ob")
                nc.vector.tensor_mul(ob, nums, rb)
                nc.gpsimd.dma_start(out=oT_d[:, qs:qs + qblk], in_=ob)

            # Per k-tile pair: emit the two ROW-TILED MM1s + exp(pair)
            # [+ boundary tri-mask], then the deferred MM2s of the previous
            # pair, so the PE stream interleaves [.. MM1s(p) MM2s(p-1) ..]
            # and fills the exp latency. Diagonal tiles (j = t - tpq*qb >= 0)
            # read/write only their live columns [128*j, qblk) in MM2.
            mm2_q = []   # deferred MM2s: (acc, pb, (t0, t1), qb, tlast)

            def epilogue_raw(acc, qb, c0, c1):
                # Last block: nothing left to hide the ~5us round-trip
                # behind. Ship raw numerators + sums; the host gather does
                # this block's divide (out[:, qs:] /= sums). Chunked by
                # column halves: cols [0:256) are final two MM2s before the
                # block ends (diag tiles j=2,3 only touch cols >= 256), so
                # the first half's copies/DMAs overlap the last matmuls.
                qs = qb * qblk + c0
                w = c1 - c0
                ssum = ob_pool.tile([1, w], f32, name=f"ssum{c0}")
                nc.vector.tensor_copy(ssum, acc[d:d + 1, c0:c1])
                nums = ob_pool.tile([d, w], f32, name=f"nums{c0}")
                nc.vector.tensor_copy(nums, acc[0:d, c0:c1])
                nc.gpsimd.dma_start(out=ns_d[:, c0:c1], in_=ssum)
                nc.gpsimd.dma_start(out=oT_d[:, qs:qs + w], in_=nums)

            def flush_mm2():
                acc_, pb_, tiles_, qb_, tlast_ = mm2_q.pop(0)
                for h, t in enumerate(tiles_):
                    j = t - tpq * qb_
                    cs = ktile * j if j > 0 else 0
                    # Last block: close CoreSim's psum accumulation-group
                    # bookkeeping at the j0 tile (stop is sim-only, no HW
                    # effect) so the chunked epilogue may read cols [0:256)
                    # while tiles j2/j3 (cols >= 256, group check skipped)
                    # are still accumulating.
                    early_stop = qb_ == nqb - 1 and t == tlast_ - 3
                    skip_chk = qb_ == nqb - 1 and t > tlast_ - 3
                    nc.tensor.matmul(
                        acc_[:, cs:],
                        lhsT=v_sb[:, t, :],
                        rhs=pb_[:, h, cs:],
                        start=(t == 0),
                        stop=(t == tlast_ or early_stop),
                        skip_group_check=skip_chk,
                    )
                if qb_ == nqb - 1 and tiles_[1] == tlast_ - 2:
                    epilogue_raw(acc_, qb_, 0, 2 * ktile)
                if tiles_[1] == tlast_:   # last pair: normalize this q-block
                    if qb_ == nqb - 1:
                        epilogue_raw(acc_, qb_, 2 * ktile, qblk)
                    else:
                        epilogue(acc_, qb_)

            for qb in range(nqb):
                ntiles = tpq * (qb + 1)
                npairs = ntiles // 2
                tlast = ntiles - 1
                acc = acc_pool.tile([d + 1, qblk], f32, name="acc", tag="acc")
                for p in range(npairs):
                    t0, t1 = 2 * p, 2 * p + 1
                    lg = lg_pool.tile([128, 2, qblk], f32, name="lg")
                    nc.tensor.matmul(
                        lg[:, 0, :],
                        lhsT=kp_sb[0:d, p, :],
                        rhs=qq_sb[0:d, qb, :],
                        start=True, stop=True,
                    )
                    nc.tensor.matmul(
                        lg[:, 1, :],
                        lhsT=kp_sb[d:2 * d, p, :],
                        rhs=qq_sb[d:2 * d, qb, :],
                        start=True, stop=True,
                    )
                    pb = pb_pool.tile([128, 2, qblk], f16, name="pb")
                    # Far-diagonal pair (tiles j=2,3): only columns >= 256
                    # are ever read by MM2 -- exp just that strided slice
                    # (one ACT op, free size 512 instead of 1024).
                    if t0 - tpq * qb == 2:
                        exp_out, exp_in = pb[:, :, 256:], lg[:, :, 256:]
                    else:
                        exp_out, exp_in = pb, lg
                    nc.scalar.activation(
                        exp_out, exp_in, mybir.ActivationFunctionType.Exp,
                        scale=scale)
                    for h, t in ((0, t0), (1, t1)):
                        j = t - tpq * qb
                        if j >= 0:
                            nc.vector.tensor_mul(
                                pb[:, h, ktile * j:ktile * (j + 1)],
                                pb[:, h, ktile * j:ktile * (j + 1)],
                                tri_sb)
                    mm2_q.append((acc, pb, (t0, t1), qb, tlast))
                    if len(mm2_q) > mm2_defer:
                        flush_mm2()
            while mm2_q:
                flush_mm2()

    nc.compile()
    return nc


def _get_nc(key="main", **kw):
    if key not in _NC_CACHE:
        _NC_CACHE[key] = build(**kw)
    return _NC_CACHE[key]


def _prep_core_inputs(q, k, v, attn_mask, b, n=N, d=D, ktile=KTILE,
                      qblk=QBLK):
    nt = n // ktile
    nqb = n // qblk
    npr = nt // 2
    kT = k[b].T.astype(np.float16)    # [d, n]
    qT = q[b].T.astype(np.float16)
    # kpair[0:64, p, :] = kT tile 2p; kpair[64:128, p, :] = kT tile 2p+1
    kpair = np.ascontiguousarray(
        kT.reshape(d, npr, 2, ktile).transpose(2, 0, 1, 3)
    ).reshape(2 * d, npr, ktile)
    # qq: qT blocks duplicated on both partition halves
    qq = np.empty((2 * d, nqb, qblk), dtype=np.float16)
    qq[0:d] = qT.reshape(d, nqb, qblk)
    qq[d:2 * d] = qq[0:d]
    v_aug = np.ones((n, d + 1), dtype=np.float32)
    v_aug[:, :d] = v[b]
    v_aug *= (attn_mask[b] != 0).astype(np.float32)[:, None]
    v_aug = np.ascontiguousarray(
        v_aug.reshape(nt, ktile, d + 1).transpose(1, 0, 2)
    ).astype(np.float16)
    # tri[kk, qc] = 1 iff qc >= kk (keep)
    tri = (np.arange(ktile)[None, :] >= np.arange(ktile)[:, None]
           ).astype(np.float16)
    return {"kpair": kpair, "qq": qq, "v_aug": v_aug, "tri": tri}


def kernel(q, k, v, attn_mask):
    global LAST_RESULTS
    q = np.asarray(q, dtype=np.float32)
    k = np.asarray(k, dtype=np.float32)
    v = np.asarray(v, dtype=np.float32)
    attn_mask = np.asarray(attn_mask)

    from concourse.bass_utils import run_bass_kernel_spmd

    nc = _get_nc()
    in_maps = [_prep_core_inputs(q, k, v, attn_mask, b) for b in range(B)]
    trace = bool(os.environ.get("BASS_TRACE"))
    last_err = None
    for attempt in range(3):
        try:
            LAST_RESULTS = run_bass_kernel_spmd(
                nc, in_maps, core_ids=list(range(B)), trace=trace)
            break
        except Exception as e:  # transient device-unrecoverable states clear
            last_err = e        # on the next execution attempt
            if "UNAVAILABLE" not in str(e) and "unrecoverable" not in str(e):
                raise
            import time as _time

            _time.sleep(2.0)
    else:
        raise last_err

    out = np.empty((B, N, D), dtype=np.float32)
    for b in range(B):
        r = LAST_RESULTS.results[b]
        oT = np.asarray(r["outT"])
        # the device ships the last q-block unnormalized + its softmax
        # denominators; finish that block's divide during the gather
        oT = oT.copy()
        oT[:, N - QBLK:] /= np.asarray(r["nsum"])[0][None, :]
        out[b] = oT.T
    return out



# revision 2
# speedup vs baseline: 1.0435x; 1.0435x over previous
"""Causal attention (B=8, N=4096, D=64) on 8 trn2 NeuronCores.

Sharding: batch b -> core b (data parallel, no cross-core comms).
Same flash-style transposed dataflow and host input layout as the
previous kernel (kpair/qq/v_aug/tri, fp16 matmuls, row-tiled MM1
pairs) -- see _prep_core_inputs. What changed and why:

The previous kernel was ACT-bound: the exp stream saturated the
Scalar engine (~68us of ACTIVATE at ~0.97ns/col) while PE/DVE/DMA
idled. This one splits the exp work across BOTH the Scalar (ACT
spline exp) and Vector (DVE) engines:

  DVE exp: a runtime-registered custom-DVE op EXP16_POLY_ANT computes
  exp(x/8) ~= ((c2*x + c1)*x + c0)^16 -- deg-2 Horner (4 ALU stages) +
  4 inline squarings (4 stages) = exactly the DVE's 8-stage pipeline,
  one instruction per tile pair, PSUM fp32 in -> SBUF fp16 out at the
  stock 1x rate ((120+FD)/0.96GHz ~ 1.19us per [128,1024] pair).
  Custom-DVE reads from PSUM measured bit-exact vs the numpy model on
  HW (exp_bisect v4). Coefficients minimax-fit exp(x/128) on the
  scaled-logit bulk [-3.5,3.5]; outside, error grows but those tails
  carry ~no softmax mass. End-to-end L2 err 2.7e-3 (vs 3.9e-4
  all-ACT), inside the 2e-2 gate with 7x margin.

  Assignment: off-diagonal pairs with even index, plus pair 1 in
  blocks qb>=5 (31 of 72) -> DVE; diagonal + far-diagonal pairs stay
  on ACT (they need tri masking / strided slices). ACT ~41 exp ops +
  copy halves ~ 41us; DVE ~31 exp ops + copy halves ~ 42us.

With exp split, the PE becomes co-critical. Each matmul pays a fixed
~173ns SBUF access latency (PE_SBUF_ACCESS_LATENCY_NS) exposed
BETWEEN instructions; concurrent row-strip matmuls hide it (v1's MM1
pairs). So every MM2 is emitted as two K=64 row-strip matmuls (keys
0-63 / 64-127 -> tile_position rows 0/64) writing acc[:, 0/1, :] in
different PSUM banks: array stream time is conserved but the strips'
fixed latencies overlap each other and the neighbouring strips
(measured PE union 61us split vs 78us unsplit at the same clock).
The strip halves are merged on the host during the gather, which
also performs the softmax divide (the device ships raw numerators +
denominator row), so the per-block epilogue is one copy acc->SBUF
(engine alternating ACT/DVE per block to balance the exp load) + one
DMA: no reciprocal, no broadcast round-trip. The far-diagonal pair's
MM1 only computes its live columns [256:). Last block ships its
epilogue in two column-halves so the copies/DMAs overlap the final
MM2s.

PSUM: lg [128,2,512] (2 banks) x3 bufs + acc [65,2,512] (2 banks) x1
buf = 8 banks. lg_bufs=3 matters: with 2, each engine's next exp
waits on an MM1 that waits on that engine's previous exp (measured
+13us). mm2_defer=2 (not 1) keeps the deferred MM2s' semaphore waits
from blocking later MM1s in the in-order PE queue -- with defer=1 the
two exp engines serialize against each other through the PE queue
(measured 100us -> 85us). acc single-buffering makes block qb+1's
first MM2s wait on block qb's epilogue copy; both are pipelined
behind the first pair of MM1/exp work of block qb+1.

Tri masking of diagonal tiles runs on GpSimd (tensor_mul on SBUF fp16)
keeping the Vector queue free for exp; GpSimd otherwise only issues the
per-block epilogue DMA descriptors. A chain of tiny matmuls in the
input-DMA dead zone carries the PE pstate ramp (1.2->2.4GHz needs
~3.4us of continuous matmul activity); the four earliest-needed input
DMAs issue on three different queues (sync/scalar/gpsimd) so their
descriptor generation overlaps.

Perf (8 cores SPMD, hot device with chip activity-throttle limiting
the PE clock to ~0.69-0.76 of max): 84.8us best, vs the previous
ACT-bound kernel's 90.5us on a RESTED device (107.7us measured hot).
The PE array is the wall: its conserved column-stream work is ~104k
cycles = 43us at full clock, 57-62us throttled; ACT/DVE exp streams
(~41us each) hide under it. ACT and DVE paces measured unaffected by
the throttle; only the PE clock drops.
"""

import os
from contextlib import ExitStack

import numpy as np

B, N, D = 8, 4096, 64
QBLK = 512
KTILE = 128

# EXP16_POLY_ANT coefficients: relative-minimax fit of exp(x/128) on
# x in [-28, 28] (deg 2).
EXP16_C0 = 1.00007108e+00
EXP16_C1 = 7.85903363e-03
EXP16_C2 = 3.04262605e-05

LAST_RESULTS = None
_NC_CACHE = {}


def _register_exp16():
    import concourse.dve_ops as dve_ops
    from concourse.dve_spec import Spec, Src0, C0, C1, C2, lower, sq
    from concourse.dve_uop import DveOpSpec

    name = "EXP16_POLY_ANT"
    for o in dve_ops.OPS:
        if o.name == name:
            return o
    x = Src0
    p = (C2 * x + C1) * x + C0
    body = sq(sq(sq(sq(p))))

    def ref(in0, in1, s0, s1, imm2):
        pp = (np.float32(imm2) * in0 + np.float32(s1)) * in0 + np.float32(s0)
        for _ in range(4):
            pp = (pp * pp).astype(np.float32)
        return pp

    spec = Spec(body=body, reference=ref)
    row = max(dve_ops._SUB_OPCODE_FOR_NAME.values()) + 1
    assert row < 0x20
    dve_ops._SUB_OPCODE_FOR_NAME[name] = row
    shas = {}
    for ver in ("v3", "v4"):
        try:
            uops = lower(spec, ver=ver)
            shas[ver] = DveOpSpec(
                name=name, opcode=row, uops=uops, rd1_en=False
            ).sha(ver)
        except Exception:
            pass
    op = dve_ops.DveOp(name, spec, subdim=False, uops_sha=shas)
    dve_ops.OPS.append(op)
    dve_ops.CUSTOM_DVE_SPECS[name] = spec
    return op


def build(n=N, d=D, qblk=QBLK, ktile=KTILE, lg_bufs=3, pb_bufs=8,
          mm2_defer=2, tri_engine="gpsimd", dve_mod=2):
    import concourse.bass as bass  # noqa: F401
    import concourse.mybir as mybir
    import concourse.tile as tile
    from concourse import bacc

    exp_op = _register_exp16()

    f32 = mybir.dt.float32
    f16 = mybir.dt.float16
    nt = n // ktile          # number of k-tiles (32)
    nqb = n // qblk          # number of q-blocks (8)
    tpq = qblk // ktile      # k-tiles per q-block (diagonal span, 4)
    npr = nt // 2            # k-tile pairs total (16)

    nc = bacc.Bacc("TRN2", target_bir_lowering=False, debug=False,
                   enable_asserts=False)

    kp_d = nc.dram_tensor("kpair", (2 * d, npr, ktile), f16,
                          kind="ExternalInput").ap()
    qq_d = nc.dram_tensor("qq", (2 * d, nqb, qblk), f16,
                          kind="ExternalInput").ap()
    v_d = nc.dram_tensor("v_aug", (ktile, nt, d + 1), f16,
                         kind="ExternalInput").ap()
    tri_d = nc.dram_tensor("tri", (ktile, ktile), f16,
                           kind="ExternalInput").ap()
    oT_d = nc.dram_tensor("outT2", (d + 1, 2, n), f32,
                          kind="ExternalOutput").ap()

    scale = 1.0 / float(np.sqrt(d))

    with tile.TileContext(nc) as tc:
        with ExitStack() as ctx:
            singles = ctx.enter_context(tc.tile_pool(name="singles", bufs=1))
            pb_pool = ctx.enter_context(tc.tile_pool(name="pb", bufs=pb_bufs))
            ob_pool = ctx.enter_context(tc.tile_pool(name="ob", bufs=4))
            lg_pool = ctx.enter_context(
                tc.tile_pool(name="lg", bufs=lg_bufs, space="PSUM"))
            acc_pool = ctx.enter_context(
                tc.tile_pool(name="acc", bufs=1, space="PSUM"))

            # --- resident inputs -------------------------------------------
            kp_sb = singles.tile([2 * d, npr, ktile], f16)
            qq_sb = singles.tile([2 * d, nqb, qblk], f16)
            v_sb = singles.tile([ktile, nt, d + 1], f16)
            tri_sb = singles.tile([ktile, ktile], f16)

            # Engine warmup: dependency-free ops pull first-use setup
            # (TENSOR_LOAD, ACT_TABLE_LOAD, custom-DVE path) into the DMA
            # dead zone at kernel start.
            nc.scalar.dma_start(out=qq_sb[:, 0, :], in_=qq_d[:, 0, :])
            nc.gpsimd.dma_start(out=tri_sb, in_=tri_d)
            wpe = ob_pool.tile([64, 64], f16, name="wpe")
            nc.gpsimd.memset(wpe, 0.0)
            warm = ob_pool.tile([1, 4], f32, name="warm")
            nc.gpsimd.memset(warm, 0.0)
            warm_o = ob_pool.tile([1, 4], f32, name="warm_o")
            nc.scalar.activation(warm_o, warm,
                                 mybir.ActivationFunctionType.Exp,
                                 scale=scale)
            warm_v = ob_pool.tile([1, 4], f32, name="warm_v")
            nc.vector._custom_dve(exp_op, out=warm_v, in0=warm,
                                  s0=EXP16_C0, s1=EXP16_C1, imm2=EXP16_C2)
            warm_g = ob_pool.tile([1, 4], f32, name="warm_g")
            nc.gpsimd.tensor_mul(warm_g, warm, warm)
            # PE pstate warmup: the PE clock ramps 1.2->2.4GHz only after
            # ~3.4us of CONTINUOUS matmul activity (and decays when idle).
            # A chain of tiny dependency-free matmuls in the input-DMA dead
            # zone carries the ramp so the first real MM1s run at speed.
            wps = acc_pool.tile([64, 64], f32, name="wps", tag="acc")
            for _ in range(24):
                nc.tensor.matmul(wps, lhsT=wpe, rhs=wpe, start=True,
                                 stop=True)

            # Input DMA order: v1's measured optimum, except the four
            # earliest-needed tensors issue on four DIFFERENT engine queues
            # so their descriptor generation runs in parallel and the first
            # MM1 starts ~1.5us sooner.
            nc.sync.dma_start(out=kp_sb[:, 0:2, :], in_=kp_d[:, 0:2, :])
            nc.sync.dma_start(out=qq_sb[:, 1, :], in_=qq_d[:, 1, :])
            nc.sync.dma_start(out=qq_sb[:, 2, :], in_=qq_d[:, 2, :])
            nc.sync.dma_start(out=v_sb[:, 0:4, :], in_=v_d[:, 0:4, :])
            nc.sync.dma_start(out=qq_sb[:, 3, :], in_=qq_d[:, 3, :])
            nc.sync.dma_start(out=kp_sb[:, 2:8, :], in_=kp_d[:, 2:8, :])
            nc.sync.dma_start(out=v_sb[:, 4:16, :], in_=v_d[:, 4:16, :])
            nc.sync.dma_start(out=qq_sb[:, 4:nqb, :], in_=qq_d[:, 4:nqb, :])
            nc.sync.dma_start(out=kp_sb[:, 8:npr, :], in_=kp_d[:, 8:npr, :])
            nc.sync.dma_start(out=v_sb[:, 16:nt, :], in_=v_d[:, 16:nt, :])

            tri_eng = nc.gpsimd if tri_engine == "gpsimd" else nc.vector

            # --- main loop -------------------------------------------------
            mm2_q = []   # deferred MM2s: (acc, pb, (t0, t1), qb, tlast)

            def emit_mm2(acc_, pb_, t, h, qb_, tlast_):
                j = t - tpq * qb_
                cs = ktile * j if j > 0 else 0
                # Last block: close CoreSim's psum accumulation-group
                # bookkeeping at the j0 tile so the chunked epilogue may
                # read cols [0:256) while j2/j3 (cols >= 256) accumulate.
                last = qb_ == nqb - 1
                early_stop = last and t == tlast_ - 3
                skip_chk = last and t > tlast_ - 3
                for st in range(2):  # K row strips: keys 0-63 / 64-127
                    nc.tensor.matmul(
                        acc_[:, st, cs:],
                        lhsT=v_sb[64 * st:64 * (st + 1), t, :],
                        rhs=pb_[64 * st:64 * (st + 1), h, cs:],
                        start=(t == 0),
                        stop=(t == tlast_ or early_stop),
                        skip_group_check=skip_chk,
                    )

            def epilogue(acc_, qb_, c0, c1):
                # acc -> SBUF (frees the single PSUM acc buffer) -> DRAM.
                # Host merges the two K-strips and divides by row d. The
                # copy engine alternates per block to balance ACT/DVE load.
                qs = qb_ * qblk + c0
                w = c1 - c0
                eb = ob_pool.tile([d + 1, 2, w], f32, name=f"eb{c0}")
                if w > 2 * ktile:
                    # split the copy across BOTH exp engines so the acc
                    # PSUM buffer frees ~2x sooner (the next block's first
                    # MM2 start is gated on it)
                    h = w // 2
                    nc.scalar.copy(eb[:, :, 0:h], acc_[:, :, c0:c0 + h])
                    nc.vector.tensor_copy(eb[:, :, h:], acc_[:, :, c0 + h:c1])
                elif (qb_ + c0 // ktile) % 2 == 0:
                    nc.scalar.copy(eb, acc_[:, :, c0:c1])
                else:
                    nc.vector.tensor_copy(eb, acc_[:, :, c0:c1])
                nc.gpsimd.dma_start(out=oT_d[:, :, qs:qs + w], in_=eb)

            def flush_mm2():
                acc_, pb_, tiles_, qb_, tlast_ = mm2_q.pop(0)
                last = qb_ == nqb - 1
                for h, t in enumerate(tiles_):
                    emit_mm2(acc_, pb_, t, h, qb_, tlast_)
                    # last block: ship each column range the moment its
                    # final contribution lands (j2 owns [256:384) end,
                    # j3 owns [384:512)) so only the smallest chunk's
                    # copy+DMA trails the last matmul.
                    if last and t == tlast_ - 1:
                        epilogue(acc_, qb_, 2 * ktile, 3 * ktile)
                if last and tiles_[1] == tlast_ - 2:
                    epilogue(acc_, qb_, 0, 2 * ktile)
                if tiles_[1] == tlast_:
                    if last:
                        epilogue(acc_, qb_, 3 * ktile, qblk)
                    else:
                        epilogue(acc_, qb_, 0, qblk)

            for qb in range(nqb):
                ntiles = tpq * (qb + 1)
                npairs = ntiles // 2
                tlast = ntiles - 1
                acc = acc_pool.tile([d + 1, 2, qblk], f32, name="acc",
                                    tag="acc")
                for p in range(npairs):
                    t0, t1 = 2 * p, 2 * p + 1
                    j0 = t0 - tpq * qb
                    # far-diagonal pair: only cols >= 256 are ever read
                    ms = 2 * ktile if j0 == 2 else 0
                    lg = lg_pool.tile([128, 2, qblk], f32, name="lg")
                    nc.tensor.matmul(
                        lg[:, 0, ms:],
                        lhsT=kp_sb[0:d, p, :],
                        rhs=qq_sb[0:d, qb, ms:],
                        start=True, stop=True,
                    )
                    nc.tensor.matmul(
                        lg[:, 1, ms:],
                        lhsT=kp_sb[d:2 * d, p, :],
                        rhs=qq_sb[d:2 * d, qb, ms:],
                        start=True, stop=True,
                    )
                    pb = pb_pool.tile([128, 2, qblk], f16, name="pb")
                    if j0 == 2:
                        # far-diagonal pair: only cols >= 256 are live
                        nc.scalar.activation(
                            pb[:, :, 256:], lg[:, :, 256:],
                            mybir.ActivationFunctionType.Exp, scale=scale)
                    elif j0 < 0 and (p % dve_mod == 0
                                     or (qb >= 5 and p == 1)):
                        # off-diagonal even pair -> DVE exp
                        nc.vector._custom_dve(
                            exp_op, out=pb, in0=lg,
                            s0=EXP16_C0, s1=EXP16_C1, imm2=EXP16_C2)
                    else:
                        nc.scalar.activation(
                            pb, lg, mybir.ActivationFunctionType.Exp,
                            scale=scale)
                    for h, t in ((0, t0), (1, t1)):
                        j = t - tpq * qb
                        if j >= 0:
                            tri_eng.tensor_mul(
                                pb[:, h, ktile * j:ktile * (j + 1)],
                                pb[:, h, ktile * j:ktile * (j + 1)],
                                tri_sb)
                    mm2_q.append((acc, pb, (t0, t1), qb, tlast))
                    if len(mm2_q) > mm2_defer:
                        flush_mm2()
            while mm2_q:
                flush_mm2()

    nc.compile()
    return nc


def _get_nc(key="main", **kw):
    if key not in _NC_CACHE:
        _NC_CACHE[key] = build(**kw)
    return _NC_CACHE[key]


def _prep_core_inputs(q, k, v, attn_mask, b, n=N, d=D, ktile=KTILE,
                      qblk=QBLK):
    nt = n // ktile
    nqb = n // qblk
    npr = nt // 2
    kT = k[b].T.astype(np.float16)    # [d, n]
    qT = q[b].T.astype(np.float16)
    kpair = np.ascontiguousarray(
        kT.reshape(d, npr, 2, ktile).transpose(2, 0, 1, 3)
    ).reshape(2 * d, npr, ktile)
    qq = np.empty((2 * d, nqb, qblk), dtype=np.float16)
    qq[0:d] = qT.reshape(d, nqb, qblk)
    qq[d:2 * d] = qq[0:d]
    v_aug = np.ones((n, d + 1), dtype=np.float32)
    v_aug[:, :d] = v[b]
    v_aug *= (attn_mask[b] != 0).astype(np.float32)[:, None]
    v_aug = np.ascontiguousarray(
        v_aug.reshape(nt, ktile, d + 1).transpose(1, 0, 2)
    ).astype(np.float16)
    tri = (np.arange(ktile)[None, :] >= np.arange(ktile)[:, None]
           ).astype(np.float16)
    return {"kpair": kpair, "qq": qq, "v_aug": v_aug, "tri": tri}


def kernel(q, k, v, attn_mask):
    global LAST_RESULTS
    q = np.asarray(q, dtype=np.float32)
    k = np.asarray(k, dtype=np.float32)
    v = np.asarray(v, dtype=np.float32)
    attn_mask = np.asarray(attn_mask)

    from concourse.bass_utils import run_bass_kernel_spmd

    nc = _get_nc()
    in_maps = [_prep_core_inputs(q, k, v, attn_mask, b) for b in range(B)]
    trace = bool(os.environ.get("BASS_TRACE"))
    last_err = None
    for attempt in range(3):
        try:
            LAST_RESULTS = run_bass_kernel_spmd(
                nc, in_maps, core_ids=list(range(B)), trace=trace)
            break
        except Exception as e:  # transient device-unrecoverable states clear
            last_err = e
            retryable = any(s in str(e) for s in
                            ("UNAVAILABLE", "unrecoverable", "INTERNAL"))
            if not retryable:
                raise
            import time as _time

            _time.sleep(2.0)
    else:
        raise last_err

    out = np.empty((B, N, D), dtype=np.float32)
    for b in range(B):
        r = np.asarray(LAST_RESULTS.results[b]["outT2"])
        m = r[:, 0, :] + r[:, 1, :]        # merge the K row-strip halves
        out[b] = (m[0:D] / m[D:D + 1]).T   # softmax divide + transpose
    return out


# revision 3
# speedup vs baseline: 1.0542x; 1.0103x over previous
"""Causal attention (B=8, N=4096, D=64) on 8 trn2 NeuronCores.

Sharding: batch b -> core b (data parallel, no cross-core comms).
Same flash-style transposed dataflow and host input layout as the
previous kernel (kpair/qq/v_aug/tri, fp16 matmuls, row-tiled MM1
pairs) -- see _prep_core_inputs. What changed and why:

The previous kernel was ACT-bound: the exp stream saturated the
Scalar engine (~68us of ACTIVATE at ~0.97ns/col) while PE/DVE/DMA
idled. This one splits the exp work across BOTH the Scalar (ACT
spline exp) and Vector (DVE) engines:

  DVE exp: a runtime-registered custom-DVE op EXP16_POLY_ANT computes
  exp(x/8) ~= ((c2*x + c1)*x + c0)^16 -- deg-2 Horner (4 ALU stages) +
  4 inline squarings (4 stages) = exactly the DVE's 8-stage pipeline,
  one instruction per tile pair, PSUM fp32 in -> SBUF fp16 out at the
  stock 1x rate ((120+FD)/0.96GHz ~ 1.19us per [128,1024] pair).
  Custom-DVE reads from PSUM measured bit-exact vs the numpy model on
  HW (exp_bisect v4). Coefficients minimax-fit exp(x/128) on the
  scaled-logit bulk [-3.5,3.5]; outside, error grows but those tails
  carry ~no softmax mass. End-to-end L2 err 2.7e-3 (vs 3.9e-4
  all-ACT), inside the 2e-2 gate with 7x margin.

  Assignment: off-diagonal pairs with even index, plus pair 1 in
  blocks qb>=5 (31 of 72) -> DVE; diagonal + far-diagonal pairs stay
  on ACT (they need tri masking / strided slices). ACT ~41 exp ops +
  copy halves ~ 41us; DVE ~31 exp ops + copy halves ~ 42us.

With exp split, the PE becomes co-critical. Each matmul pays a fixed
~173ns SBUF access latency (PE_SBUF_ACCESS_LATENCY_NS) exposed
BETWEEN instructions; concurrent row-strip matmuls hide it (v1's MM1
pairs). So every MM2 is emitted as two K=64 row-strip matmuls (keys
0-63 / 64-127 -> tile_position rows 0/64) writing acc[:, 0/1, :] in
different PSUM banks: array stream time is conserved but the strips'
fixed latencies overlap each other and the neighbouring strips
(measured PE union 61us split vs 78us unsplit at the same clock).
The strip halves are merged on the host during the gather, which
also performs the softmax divide (the device ships raw numerators +
denominator row), so the per-block epilogue is one copy acc->SBUF
(engine alternating ACT/DVE per block to balance the exp load) + one
DMA: no reciprocal, no broadcast round-trip. The far-diagonal pair's
MM1 only computes its live columns [256:). Last block ships its
epilogue in two column-halves so the copies/DMAs overlap the final
MM2s.

PSUM: lg [128,2,512] (2 banks) x3 bufs + acc [65,2,512] (2 banks) x1
buf = 8 banks. lg_bufs=3 matters: with 2, each engine's next exp
waits on an MM1 that waits on that engine's previous exp (measured
+13us). mm2_defer=2 (not 1) keeps the deferred MM2s' semaphore waits
from blocking later MM1s in the in-order PE queue -- with defer=1 the
two exp engines serialize against each other through the PE queue
(measured 100us -> 85us). acc single-buffering makes block qb+1's
first MM2s wait on block qb's epilogue copy; both are pipelined
behind the first pair of MM1/exp work of block qb+1.

Tri masking of diagonal tiles runs on GpSimd (tensor_mul on SBUF fp16)
keeping the Vector queue free for exp; GpSimd otherwise only issues the
per-block epilogue DMA descriptors. A chain of tiny matmuls in the
input-DMA dead zone carries the PE pstate ramp (1.2->2.4GHz needs
~3.4us of continuous matmul activity); the four earliest-needed input
DMAs issue on three different queues (sync/scalar/gpsimd) so their
descriptor generation overlaps.

Perf (8 cores SPMD, hot device with chip activity-throttle limiting
the PE clock to ~0.69-0.76 of max): 84.8us best, vs the previous
ACT-bound kernel's 90.5us on a RESTED device (107.7us measured hot).
The PE array is the wall: its conserved column-stream work is ~104k
cycles = 43us at full clock, 57-62us throttled; ACT/DVE exp streams
(~41us each) hide under it. ACT and DVE paces measured unaffected by
the throttle; only the PE clock drops.
"""

import os
from contextlib import ExitStack

import numpy as np

B, N, D = 8, 4096, 64
QBLK = 512
KTILE = 128

# EXP16_POLY_ANT coefficients: relative-minimax fit of exp(x/128) on
# x in [-28, 28] (deg 2).
EXP16_C0 = 1.00007108e+00
EXP16_C1 = 7.85903363e-03
EXP16_C2 = 3.04262605e-05

LAST_RESULTS = None
_NC_CACHE = {}


def _register_exp16():
    import concourse.dve_ops as dve_ops
    from concourse.dve_spec import Spec, Src0, C0, C1, C2, lower, sq
    from concourse.dve_uop import DveOpSpec

    name = "EXP16_POLY_ANT"
    for o in dve_ops.OPS:
        if o.name == name:
            return o
    x = Src0
    p = (C2 * x + C1) * x + C0
    body = sq(sq(sq(sq(p))))

    def ref(in0, in1, s0, s1, imm2):
        pp = (np.float32(imm2) * in0 + np.float32(s1)) * in0 + np.float32(s0)
        for _ in range(4):
            pp = (pp * pp).astype(np.float32)
        return pp

    spec = Spec(body=body, reference=ref)
    row = max(dve_ops._SUB_OPCODE_FOR_NAME.values()) + 1
    assert row < 0x20
    dve_ops._SUB_OPCODE_FOR_NAME[name] = row
    shas = {}
    for ver in ("v3", "v4"):
        try:
            uops = lower(spec, ver=ver)
            shas[ver] = DveOpSpec(
                name=name, opcode=row, uops=uops, rd1_en=False
            ).sha(ver)
        except Exception:
            pass
    op = dve_ops.DveOp(name, spec, subdim=False, uops_sha=shas)
    dve_ops.OPS.append(op)
    dve_ops.CUSTOM_DVE_SPECS[name] = spec
    return op


def build(n=N, d=D, qblk=QBLK, ktile=KTILE, lg_bufs=3, pb_bufs=8,
          mm2_defer=2, tri_engine="gpsimd", dve_mod=2):
    import concourse.bass as bass  # noqa: F401
    import concourse.mybir as mybir
    import concourse.tile as tile
    from concourse import bacc

    exp_op = _register_exp16()

    f32 = mybir.dt.float32
    f16 = mybir.dt.float16
    nt = n // ktile          # number of k-tiles (32)
    nqb = n // qblk          # number of q-blocks (8)
    tpq = qblk // ktile      # k-tiles per q-block (diagonal span, 4)
    npr = nt // 2            # k-tile pairs total (16)

    nc = bacc.Bacc("TRN2", target_bir_lowering=False, debug=False,
                   enable_asserts=False)

    kp_d = nc.dram_tensor("kpair", (2 * d, npr, ktile), f16,
                          kind="ExternalInput").ap()
    qq_d = nc.dram_tensor("qq", (2 * d, nqb, qblk), f16,
                          kind="ExternalInput").ap()
    v_d = nc.dram_tensor("v_aug", (ktile, nt, d + 1), f16,
                         kind="ExternalInput").ap()
    tri_d = nc.dram_tensor("tri", (ktile, ktile), f16,
                           kind="ExternalInput").ap()
    oT_d = nc.dram_tensor("outT2", (d + 1, 2, n), f32,
                          kind="ExternalOutput").ap()

    scale = 1.0 / float(np.sqrt(d))

    with tile.TileContext(nc) as tc:
        with ExitStack() as ctx:
            singles = ctx.enter_context(tc.tile_pool(name="singles", bufs=1))
            pb_pool = ctx.enter_context(tc.tile_pool(name="pb", bufs=pb_bufs))
            ob_pool = ctx.enter_context(tc.tile_pool(name="ob", bufs=4))
            lg_pool = ctx.enter_context(
                tc.tile_pool(name="lg", bufs=lg_bufs, space="PSUM"))
            acc_pool = ctx.enter_context(
                tc.tile_pool(name="acc", bufs=1, space="PSUM"))

            # --- resident inputs -------------------------------------------
            kp_sb = singles.tile([2 * d, npr, ktile], f16)
            qq_sb = singles.tile([2 * d, nqb, qblk], f16)
            v_sb = singles.tile([ktile, nt, d + 1], f16)
            tri_sb = singles.tile([ktile, ktile], f16)

            # Engine warmup: dependency-free ops pull first-use setup
            # (TENSOR_LOAD, ACT_TABLE_LOAD, custom-DVE path) into the DMA
            # dead zone at kernel start.
            nc.scalar.dma_start(out=qq_sb[:, 0, :], in_=qq_d[:, 0, :])
            nc.gpsimd.dma_start(out=tri_sb, in_=tri_d)
            wpe = ob_pool.tile([64, 64], f16, name="wpe")
            nc.gpsimd.memset(wpe, 0.0)
            warm = ob_pool.tile([1, 4], f32, name="warm")
            nc.gpsimd.memset(warm, 0.0)
            warm_o = ob_pool.tile([1, 4], f32, name="warm_o")
            nc.scalar.activation(warm_o, warm,
                                 mybir.ActivationFunctionType.Exp,
                                 scale=scale)
            warm_v = ob_pool.tile([1, 4], f32, name="warm_v")
            nc.vector._custom_dve(exp_op, out=warm_v, in0=warm,
                                  s0=EXP16_C0, s1=EXP16_C1, imm2=EXP16_C2)
            warm_g = ob_pool.tile([1, 4], f32, name="warm_g")
            nc.gpsimd.tensor_mul(warm_g, warm, warm)
            # PE pstate warmup: the PE clock ramps 1.2->2.4GHz only after
            # ~3.4us of CONTINUOUS matmul activity (and decays when idle).
            # A chain of tiny dependency-free matmuls in the input-DMA dead
            # zone carries the ramp so the first real MM1s run at speed.
            wps = acc_pool.tile([64, 64], f32, name="wps", tag="acc")
            for _ in range(24):
                nc.tensor.matmul(wps, lhsT=wpe, rhs=wpe, start=True,
                                 stop=True)

            # Input DMA order: v1's measured optimum, except the four
            # earliest-needed tensors issue on four DIFFERENT engine queues
            # so their descriptor generation runs in parallel and the first
            # MM1 starts ~1.5us sooner.
            nc.sync.dma_start(out=kp_sb[:, 0:2, :], in_=kp_d[:, 0:2, :])
            nc.sync.dma_start(out=qq_sb[:, 1, :], in_=qq_d[:, 1, :])
            nc.sync.dma_start(out=qq_sb[:, 2, :], in_=qq_d[:, 2, :])
            nc.sync.dma_start(out=v_sb[:, 0:4, :], in_=v_d[:, 0:4, :])
            nc.sync.dma_start(out=qq_sb[:, 3, :], in_=qq_d[:, 3, :])
            nc.sync.dma_start(out=kp_sb[:, 2:8, :], in_=kp_d[:, 2:8, :])
            nc.sync.dma_start(out=v_sb[:, 4:16, :], in_=v_d[:, 4:16, :])
            nc.sync.dma_start(out=qq_sb[:, 4:nqb, :], in_=qq_d[:, 4:nqb, :])
            nc.sync.dma_start(out=kp_sb[:, 8:npr, :], in_=kp_d[:, 8:npr, :])
            nc.sync.dma_start(out=v_sb[:, 16:nt, :], in_=v_d[:, 16:nt, :])

            tri_eng = nc.gpsimd if tri_engine == "gpsimd" else nc.vector

            # --- main loop -------------------------------------------------
            mm2_q = []   # deferred MM2s: (acc, pb, (t0, t1), qb, tlast)

            def emit_mm2(acc_, pb_, t, h, qb_, tlast_):
                j = t - tpq * qb_
                cs = ktile * j if j > 0 else 0
                # Last block: close CoreSim's psum accumulation-group
                # bookkeeping at the j0 tile so the chunked epilogue may
                # read cols [0:256) while j2/j3 (cols >= 256) accumulate.
                last = qb_ == nqb - 1
                early_stop = last and t == tlast_ - 3
                skip_chk = last and t > tlast_ - 3
                for st in range(2):  # K row strips: keys 0-63 / 64-127
                    nc.tensor.matmul(
                        acc_[:, st, cs:],
                        lhsT=v_sb[64 * st:64 * (st + 1), t, :],
                        rhs=pb_[64 * st:64 * (st + 1), h, cs:],
                        start=(t == 0),
                        stop=(t == tlast_ or early_stop),
                        skip_group_check=skip_chk,
                    )

            def epilogue(acc_, qb_, c0, c1):
                # acc -> SBUF (frees the single PSUM acc buffer) -> DRAM.
                # Host merges the two K-strips and divides by row d. The
                # copy engine alternates per block to balance ACT/DVE load.
                qs = qb_ * qblk + c0
                w = c1 - c0
                eb = ob_pool.tile([d + 1, 2, w], f32, name=f"eb{c0}")
                if w > 2 * ktile and qb_ >= 4:
                    # boundaries into blocks 5-7: the next block's first TWO
                    # exps are both on DVE, leaving ACT idle -- both copy
                    # halves go there and run immediately (kills the ~1us
                    # PE stall waiting for the acc buffer at t=44/57/69)
                    h = w // 2
                    nc.scalar.copy(eb[:, :, 0:h], acc_[:, :, c0:c0 + h])
                    nc.scalar.copy(eb[:, :, h:], acc_[:, :, c0 + h:c1])
                elif w > 2 * ktile:
                    # split the copy across BOTH exp engines so the acc
                    # PSUM buffer frees ~2x sooner (the next block's first
                    # MM2 start is gated on it)
                    h = w // 2
                    nc.scalar.copy(eb[:, :, 0:h], acc_[:, :, c0:c0 + h])
                    nc.vector.tensor_copy(eb[:, :, h:], acc_[:, :, c0 + h:c1])
                elif (qb_ + c0 // ktile) % 2 == 0:
                    nc.scalar.copy(eb, acc_[:, :, c0:c1])
                else:
                    nc.vector.tensor_copy(eb, acc_[:, :, c0:c1])
                nc.gpsimd.dma_start(out=oT_d[:, :, qs:qs + w], in_=eb)

            def flush_mm2():
                acc_, pb_, tiles_, qb_, tlast_ = mm2_q.pop(0)
                last = qb_ == nqb - 1
                for h, t in enumerate(tiles_):
                    emit_mm2(acc_, pb_, t, h, qb_, tlast_)
                    # last block: ship each column range the moment its
                    # final contribution lands (j2 owns [256:384) end,
                    # j3 owns [384:512)) so only the smallest chunk's
                    # copy+DMA trails the last matmul.
                    if last and t == tlast_ - 1:
                        epilogue(acc_, qb_, 2 * ktile, 3 * ktile)
                if last and tiles_[1] == tlast_ - 2:
                    epilogue(acc_, qb_, 0, 2 * ktile)
                if tiles_[1] == tlast_:
                    if last:
                        epilogue(acc_, qb_, 3 * ktile, qblk)
                    else:
                        epilogue(acc_, qb_, 0, qblk)

            for qb in range(nqb):
                ntiles = tpq * (qb + 1)
                npairs = ntiles // 2
                tlast = ntiles - 1
                acc = acc_pool.tile([d + 1, 2, qblk], f32, name="acc",
                                    tag="acc")
                for p in range(npairs):
                    t0, t1 = 2 * p, 2 * p + 1
                    j0 = t0 - tpq * qb
                    # far-diagonal pair: only cols >= 256 are ever read
                    ms = 2 * ktile if j0 == 2 else 0
                    lg = lg_pool.tile([128, 2, qblk], f32, name="lg")
                    nc.tensor.matmul(
                        lg[:, 0, ms:],
                        lhsT=kp_sb[0:d, p, :],
                        rhs=qq_sb[0:d, qb, ms:],
                        start=True, stop=True,
                    )
                    nc.tensor.matmul(
                        lg[:, 1, ms:],
                        lhsT=kp_sb[d:2 * d, p, :],
                        rhs=qq_sb[d:2 * d, qb, ms:],
                        start=True, stop=True,
                    )
                    pb = pb_pool.tile([128, 2, qblk], f16, name="pb")
                    if j0 == 2:
                        # far-diagonal pair: only cols >= 256 are live
                        nc.scalar.activation(
                            pb[:, :, 256:], lg[:, :, 256:],
                            mybir.ActivationFunctionType.Exp, scale=scale)
                    elif j0 < 0 and (p % dve_mod == 0
                                     or (qb >= 5 and p == 1)):
                        # off-diagonal even pair -> DVE exp
                        nc.vector._custom_dve(
                            exp_op, out=pb, in0=lg,
                            s0=EXP16_C0, s1=EXP16_C1, imm2=EXP16_C2)
                    else:
                        nc.scalar.activation(
                            pb, lg, mybir.ActivationFunctionType.Exp,
                            scale=scale)
                    for h, t in ((0, t0), (1, t1)):
                        j = t - tpq * qb
                        if j >= 0:
                            tri_eng.tensor_mul(
                                pb[:, h, ktile * j:ktile * (j + 1)],
                                pb[:, h, ktile * j:ktile * (j + 1)],
                                tri_sb)
                    mm2_q.append((acc, pb, (t0, t1), qb, tlast))
                    if len(mm2_q) > mm2_defer:
                        flush_mm2()
            while mm2_q:
                flush_mm2()

    nc.compile()
    return nc


def _get_nc(key="main", **kw):
    if key not in _NC_CACHE:
        _NC_CACHE[key] = build(**kw)
    return _NC_CACHE[key]


def _prep_core_inputs(q, k, v, attn_mask, b, n=N, d=D, ktile=KTILE,
                      qblk=QBLK):
    nt = n // ktile
    nqb = n // qblk
    npr = nt // 2
    kT = k[b].T.astype(np.float16)    # [d, n]
    qT = q[b].T.astype(np.float16)
    kpair = np.ascontiguousarray(
        kT.reshape(d, npr, 2, ktile).transpose(2, 0, 1, 3)
    ).reshape(2 * d, npr, ktile)
    qq = np.empty((2 * d, nqb, qblk), dtype=np.float16)
    qq[0:d] = qT.reshape(d, nqb, qblk)
    qq[d:2 * d] = qq[0:d]
    v_aug = np.ones((n, d + 1), dtype=np.float32)
    v_aug[:, :d] = v[b]
    v_aug *= (attn_mask[b] != 0).astype(np.float32)[:, None]
    v_aug = np.ascontiguousarray(
        v_aug.reshape(nt, ktile, d + 1).transpose(1, 0, 2)
    ).astype(np.float16)
    tri = (np.arange(ktile)[None, :] >= np.arange(ktile)[:, None]
           ).astype(np.float16)
    return {"kpair": kpair, "qq": qq, "v_aug": v_aug, "tri": tri}


def kernel(q, k, v, attn_mask):
    global LAST_RESULTS
    q = np.asarray(q, dtype=np.float32)
    k = np.asarray(k, dtype=np.float32)
    v = np.asarray(v, dtype=np.float32)
    attn_mask = np.asarray(attn_mask)

    from concourse.bass_utils import run_bass_kernel_spmd

    nc = _get_nc()
    in_maps = [_prep_core_inputs(q, k, v, attn_mask, b) for b in range(B)]
    trace = bool(os.environ.get("BASS_TRACE"))
    last_err = None
    for attempt in range(3):
        try:
            LAST_RESULTS = run_bass_kernel_spmd(
                nc, in_maps, core_ids=list(range(B)), trace=trace)
            break
        except Exception as e:  # transient device-unrecoverable states clear
            last_err = e
            retryable = any(s in str(e) for s in
                            ("UNAVAILABLE", "unrecoverable", "INTERNAL"))
            if not retryable:
                raise
            import time as _time

            _time.sleep(2.0)
    else:
        raise last_err

    out = np.empty((B, N, D), dtype=np.float32)
    for b in range(B):
        r = np.asarray(LAST_RESULTS.results[b]["outT2"])
        m = r[:, 0, :] + r[:, 1, :]        # merge the K row-strip halves
        out[b] = (m[0:D] / m[D:D + 1]).T   # softmax divide + transpose
    return out


# revision 4
# speedup vs baseline: 1.0725x; 1.0173x over previous
"""Causal attention (B=8, N=4096, D=64) on 8 trn2 NeuronCores.

Sharding: batch b -> core b (data parallel, no cross-core comms).
Same flash-style transposed dataflow and host input layout as the
previous kernel (kpair/qq/v_aug/tri, fp16 matmuls, row-tiled MM1
pairs) -- see _prep_core_inputs. What changed and why:

The previous kernel was ACT-bound: the exp stream saturated the
Scalar engine (~68us of ACTIVATE at ~0.97ns/col) while PE/DVE/DMA
idled. This one splits the exp work across BOTH the Scalar (ACT
spline exp) and Vector (DVE) engines:

  DVE exp: a runtime-registered custom-DVE op EXP16_POLY_ANT computes
  exp(x/8) ~= ((c2*x + c1)*x + c0)^16 -- deg-2 Horner (4 ALU stages) +
  4 inline squarings (4 stages) = exactly the DVE's 8-stage pipeline,
  one instruction per tile pair, PSUM fp32 in -> SBUF fp16 out at the
  stock 1x rate ((120+FD)/0.96GHz ~ 1.19us per [128,1024] pair).
  Custom-DVE reads from PSUM measured bit-exact vs the numpy model on
  HW (exp_bisect v4). Coefficients minimax-fit exp(x/128) on the
  scaled-logit bulk [-3.5,3.5]; outside, error grows but those tails
  carry ~no softmax mass. End-to-end L2 err 2.7e-3 (vs 3.9e-4
  all-ACT), inside the 2e-2 gate with 7x margin.

  Assignment: off-diagonal pairs with even index, plus pair 1 in
  blocks qb>=5 (31 of 72) -> DVE; diagonal + far-diagonal pairs stay
  on ACT (they need tri masking / strided slices). ACT ~41 exp ops +
  copy halves ~ 41us; DVE ~31 exp ops + copy halves ~ 42us.

With exp split, the PE becomes co-critical. Each matmul pays a fixed
~173ns SBUF access latency (PE_SBUF_ACCESS_LATENCY_NS) exposed
BETWEEN instructions; concurrent row-strip matmuls hide it (v1's MM1
pairs). So every MM2 is emitted as two K=64 row-strip matmuls (keys
0-63 / 64-127 -> tile_position rows 0/64) writing acc[:, 0/1, :] in
different PSUM banks: array stream time is conserved but the strips'
fixed latencies overlap each other and the neighbouring strips
(measured PE union 61us split vs 78us unsplit at the same clock).
The strip halves are merged on the host during the gather, which
also performs the softmax divide (the device ships raw numerators +
denominator row), so the per-block epilogue is one copy acc->SBUF
(engine alternating ACT/DVE per block to balance the exp load) + one
DMA: no reciprocal, no broadcast round-trip. The far-diagonal pair's
MM1 only computes its live columns [256:). Last block ships its
epilogue in two column-halves so the copies/DMAs overlap the final
MM2s.

PSUM: lg [128,2,512] (2 banks) x3 bufs + acc [65,2,512] (2 banks) x1
buf = 8 banks. lg_bufs=3 matters: with 2, each engine's next exp
waits on an MM1 that waits on that engine's previous exp (measured
+13us). mm2_defer=2 (not 1) keeps the deferred MM2s' semaphore waits
from blocking later MM1s in the in-order PE queue -- with defer=1 the
two exp engines serialize against each other through the PE queue
(measured 100us -> 85us). acc single-buffering makes block qb+1's
first MM2s wait on block qb's epilogue copy; both are pipelined
behind the first pair of MM1/exp work of block qb+1.

Tri masking of diagonal tiles runs on GpSimd (tensor_mul on SBUF fp16)
keeping the Vector queue free for exp; GpSimd otherwise only issues the
per-block epilogue DMA descriptors. A chain of tiny matmuls in the
input-DMA dead zone carries the PE pstate ramp (1.2->2.4GHz needs
~3.4us of continuous matmul activity); the four earliest-needed input
DMAs issue on three different queues (sync/scalar/gpsimd) so their
descriptor generation overlaps.

Perf (8 cores SPMD, hot device with chip activity-throttle limiting
the PE clock to ~0.69-0.76 of max): 84.8us best, vs the previous
ACT-bound kernel's 90.5us on a RESTED device (107.7us measured hot).
The PE array is the wall: its conserved column-stream work is ~104k
cycles = 43us at full clock, 57-62us throttled; ACT/DVE exp streams
(~41us each) hide under it. ACT and DVE paces measured unaffected by
the throttle; only the PE clock drops.
"""

import os
from contextlib import ExitStack

import numpy as np

B, N, D = 8, 4096, 64
QBLK = 512
KTILE = 128

# EXP16_POLY_ANT coefficients: relative-minimax fit of exp(x/128) on
# x in [-28, 28] (deg 2).
EXP16_C0 = 1.00007108e+00
EXP16_C1 = 7.85903363e-03
EXP16_C2 = 3.04262605e-05

LAST_RESULTS = None
_NC_CACHE = {}


def _register_exp16():
    import concourse.dve_ops as dve_ops
    from concourse.dve_spec import Spec, Src0, C0, C1, C2, lower, sq
    from concourse.dve_uop import DveOpSpec

    name = "EXP16_POLY_ANT"
    for o in dve_ops.OPS:
        if o.name == name:
            return o
    x = Src0
    p = (C2 * x + C1) * x + C0
    body = sq(sq(sq(sq(p))))

    def ref(in0, in1, s0, s1, imm2):
        pp = (np.float32(imm2) * in0 + np.float32(s1)) * in0 + np.float32(s0)
        for _ in range(4):
            pp = (pp * pp).astype(np.float32)
        return pp

    spec = Spec(body=body, reference=ref)
    row = max(dve_ops._SUB_OPCODE_FOR_NAME.values()) + 1
    assert row < 0x20
    dve_ops._SUB_OPCODE_FOR_NAME[name] = row
    shas = {}
    for ver in ("v3", "v4"):
        try:
            uops = lower(spec, ver=ver)
            shas[ver] = DveOpSpec(
                name=name, opcode=row, uops=uops, rd1_en=False
            ).sha(ver)
        except Exception:
            pass
    op = dve_ops.DveOp(name, spec, subdim=False, uops_sha=shas)
    dve_ops.OPS.append(op)
    dve_ops.CUSTOM_DVE_SPECS[name] = spec
    return op


def build(n=N, d=D, qblk=QBLK, ktile=KTILE, lg_bufs=3, pb_bufs=8,
          mm2_defer=2, tri_engine="gpsimd", dve_mod=2):
    import concourse.bass as bass  # noqa: F401
    import concourse.mybir as mybir
    import concourse.tile as tile
    from concourse import bacc

    exp_op = _register_exp16()

    f32 = mybir.dt.float32
    f16 = mybir.dt.float16
    nt = n // ktile          # number of k-tiles (32)
    nqb = n // qblk          # number of q-blocks (8)
    tpq = qblk // ktile      # k-tiles per q-block (diagonal span, 4)
    npr = nt // 2            # k-tile pairs total (16)

    nc = bacc.Bacc("TRN2", target_bir_lowering=False, debug=False,
                   enable_asserts=False)

    kp_d = nc.dram_tensor("kpair", (2 * d, npr, ktile), f16,
                          kind="ExternalInput").ap()
    qq_d = nc.dram_tensor("qq", (2 * d, nqb, qblk), f16,
                          kind="ExternalInput").ap()
    v_d = nc.dram_tensor("v_aug", (ktile, nt, d + 1), f16,
                         kind="ExternalInput").ap()
    tri_d = nc.dram_tensor("tri", (ktile, ktile), f16,
                           kind="ExternalInput").ap()
    oT_d = nc.dram_tensor("outT2", (d + 1, 2, n), f32,
                          kind="ExternalOutput").ap()

    scale = 1.0 / float(np.sqrt(d))

    with tile.TileContext(nc) as tc:
        with ExitStack() as ctx:
            singles = ctx.enter_context(tc.tile_pool(name="singles", bufs=1))
            pb_pool = ctx.enter_context(tc.tile_pool(name="pb", bufs=pb_bufs))
            ob_pool = ctx.enter_context(tc.tile_pool(name="ob", bufs=4))
            lg_pool = ctx.enter_context(
                tc.tile_pool(name="lg", bufs=lg_bufs, space="PSUM"))
            acc_pool = ctx.enter_context(
                tc.tile_pool(name="acc", bufs=1, space="PSUM"))

            # --- resident inputs -------------------------------------------
            kp_sb = singles.tile([2 * d, npr, ktile], f16)
            qq_sb = singles.tile([2 * d, nqb, qblk], f16)
            v_sb = singles.tile([ktile, nt, d + 1], f16)
            tri_sb = singles.tile([ktile, ktile], f16)

            # Engine warmup: dependency-free ops pull first-use setup
            # (TENSOR_LOAD, ACT_TABLE_LOAD, custom-DVE path) into the DMA
            # dead zone at kernel start.
            nc.scalar.dma_start(out=qq_sb[:, 0, :], in_=qq_d[:, 0, :])
            nc.gpsimd.dma_start(out=tri_sb, in_=tri_d)
            wpe = ob_pool.tile([64, 64], f16, name="wpe")
            nc.gpsimd.memset(wpe, 0.0)
            warm = ob_pool.tile([1, 4], f32, name="warm")
            nc.gpsimd.memset(warm, 0.0)
            warm_o = ob_pool.tile([1, 4], f32, name="warm_o")
            nc.scalar.activation(warm_o, warm,
                                 mybir.ActivationFunctionType.Exp,
                                 scale=scale)
            warm_v = ob_pool.tile([1, 4], f32, name="warm_v")
            nc.vector._custom_dve(exp_op, out=warm_v, in0=warm,
                                  s0=EXP16_C0, s1=EXP16_C1, imm2=EXP16_C2)
            warm_g = ob_pool.tile([1, 4], f32, name="warm_g")
            nc.gpsimd.tensor_mul(warm_g, warm, warm)
            # PE pstate warmup: the PE clock ramps 1.2->2.4GHz only after
            # ~3.4us of CONTINUOUS matmul activity (and decays when idle).
            # A chain of tiny dependency-free matmuls in the input-DMA dead
            # zone carries the ramp so the first real MM1s run at speed.
            wps = acc_pool.tile([64, 64], f32, name="wps", tag="acc")
            for _ in range(24):
                nc.tensor.matmul(wps, lhsT=wpe, rhs=wpe, start=True,
                                 stop=True)

            # Input DMA order: v1's measured optimum, except the four
            # earliest-needed tensors issue on four DIFFERENT engine queues
            # so their descriptor generation runs in parallel and the first
            # MM1 starts ~1.5us sooner.
            nc.sync.dma_start(out=kp_sb[:, 0:2, :], in_=kp_d[:, 0:2, :])
            nc.sync.dma_start(out=qq_sb[:, 1, :], in_=qq_d[:, 1, :])
            nc.sync.dma_start(out=qq_sb[:, 2, :], in_=qq_d[:, 2, :])
            nc.sync.dma_start(out=v_sb[:, 0:4, :], in_=v_d[:, 0:4, :])
            nc.sync.dma_start(out=qq_sb[:, 3, :], in_=qq_d[:, 3, :])
            nc.sync.dma_start(out=kp_sb[:, 2:8, :], in_=kp_d[:, 2:8, :])
            nc.sync.dma_start(out=v_sb[:, 4:16, :], in_=v_d[:, 4:16, :])
            nc.sync.dma_start(out=qq_sb[:, 4:nqb, :], in_=qq_d[:, 4:nqb, :])
            nc.sync.dma_start(out=kp_sb[:, 8:npr, :], in_=kp_d[:, 8:npr, :])
            nc.sync.dma_start(out=v_sb[:, 16:nt, :], in_=v_d[:, 16:nt, :])

            tri_eng = nc.gpsimd if tri_engine == "gpsimd" else nc.vector

            # --- main loop -------------------------------------------------
            mm2_q = []   # deferred MM2s: (acc, pb, (t0, t1), qb, tlast)

            def emit_mm2(acc_, pb_, t, h, qb_, tlast_):
                j = t - tpq * qb_
                cs = ktile * j if j > 0 else 0
                # Last block: close CoreSim's psum accumulation-group
                # bookkeeping at the j0 tile so the chunked epilogue may
                # read cols [0:256) while j2/j3 (cols >= 256) accumulate.
                last = qb_ == nqb - 1
                early_stop = last and t == tlast_ - 3
                skip_chk = last and t > tlast_ - 3
                for st in range(2):  # K row strips: keys 0-63 / 64-127
                    nc.tensor.matmul(
                        acc_[:, st, cs:],
                        lhsT=v_sb[64 * st:64 * (st + 1), t, :],
                        rhs=pb_[64 * st:64 * (st + 1), h, cs:],
                        start=(t == 0),
                        stop=(t == tlast_ or early_stop),
                        skip_group_check=skip_chk,
                    )

            def epilogue(acc_, qb_, c0, c1):
                # acc -> SBUF (frees the single PSUM acc buffer) -> DRAM.
                # Host merges the two K-strips and divides by row d. The
                # copy engine alternates per block to balance ACT/DVE load.
                qs = qb_ * qblk + c0
                w = c1 - c0
                eb = ob_pool.tile([d + 1, 2, w], f32, name=f"eb{c0}")
                if w > 2 * ktile and qb_ >= 4:
                    # boundaries into blocks 5-7: the next block's first TWO
                    # exps are both on DVE, leaving ACT idle -- both copy
                    # halves go there and run immediately (kills the ~1us
                    # PE stall waiting for the acc buffer at t=44/57/69)
                    h = w // 2
                    nc.scalar.copy(eb[:, :, 0:h], acc_[:, :, c0:c0 + h])
                    nc.scalar.copy(eb[:, :, h:], acc_[:, :, c0 + h:c1])
                elif w > 2 * ktile:
                    # split the copy across BOTH exp engines so the acc
                    # PSUM buffer frees ~2x sooner (the next block's first
                    # MM2 start is gated on it)
                    h = w // 2
                    nc.scalar.copy(eb[:, :, 0:h], acc_[:, :, c0:c0 + h])
                    nc.vector.tensor_copy(eb[:, :, h:], acc_[:, :, c0 + h:c1])
                elif (qb_ + c0 // ktile) % 2 == 0:
                    nc.scalar.copy(eb, acc_[:, :, c0:c1])
                else:
                    nc.vector.tensor_copy(eb, acc_[:, :, c0:c1])
                nc.gpsimd.dma_start(out=oT_d[:, :, qs:qs + w], in_=eb)

            def flush_mm2():
                acc_, pb_, tiles_, qb_, tlast_ = mm2_q.pop(0)
                last = qb_ == nqb - 1
                for h, t in enumerate(tiles_):
                    emit_mm2(acc_, pb_, t, h, qb_, tlast_)
                    # last block: ship each column range the moment its
                    # final contribution lands (j2 owns [256:384) end,
                    # j3 owns [384:512)) so only the smallest chunk's
                    # copy+DMA trails the last matmul.
                    if last and t == tlast_ - 1:
                        epilogue(acc_, qb_, 2 * ktile, 3 * ktile)
                if last and tiles_[1] == tlast_ - 2:
                    epilogue(acc_, qb_, 0, 2 * ktile)
                if tiles_[1] == tlast_:
                    if last:
                        epilogue(acc_, qb_, 3 * ktile, qblk)
                    else:
                        epilogue(acc_, qb_, 0, qblk)

            for qb in range(nqb):
                ntiles = tpq * (qb + 1)
                npairs = ntiles // 2
                tlast = ntiles - 1
                acc = acc_pool.tile([d + 1, 2, qblk], f32, name="acc",
                                    tag="acc")
                for p in range(npairs):
                    t0, t1 = 2 * p, 2 * p + 1
                    j0 = t0 - tpq * qb
                    # far-diagonal pair: only cols >= 256 are ever read
                    ms = 2 * ktile if j0 == 2 else 0
                    lg = lg_pool.tile([128, 2, qblk], f32, name="lg")
                    nc.tensor.matmul(
                        lg[:, 0, ms:],
                        lhsT=kp_sb[0:d, p, :],
                        rhs=qq_sb[0:d, qb, ms:],
                        start=True, stop=True,
                    )
                    nc.tensor.matmul(
                        lg[:, 1, ms:],
                        lhsT=kp_sb[d:2 * d, p, :],
                        rhs=qq_sb[d:2 * d, qb, ms:],
                        start=True, stop=True,
                    )
                    pb = pb_pool.tile([128, 2, qblk], f16, name="pb")
                    if j0 == 2:
                        # far-diagonal pair: only cols >= 256 are live
                        nc.scalar.activation(
                            pb[:, :, 256:], lg[:, :, 256:],
                            mybir.ActivationFunctionType.Exp, scale=scale)
                    elif j0 < 0 and (p % dve_mod == 0
                                     or (qb >= 5 and p == 1)):
                        # off-diagonal even pair -> DVE exp
                        nc.vector._custom_dve(
                            exp_op, out=pb, in0=lg,
                            s0=EXP16_C0, s1=EXP16_C1, imm2=EXP16_C2)
                    else:
                        nc.scalar.activation(
                            pb, lg, mybir.ActivationFunctionType.Exp,
                            scale=scale)
                    for h, t in ((0, t0), (1, t1)):
                        j = t - tpq * qb
                        if j >= 0:
                            # last block: tri on the Vector engine (idle by
                            # then, 3x faster per mul than GpSimd) -- cuts
                            # the exp->tri->MM2 tail chain. Earlier blocks
                            # keep GpSimd so tri never blocks DVE exps.
                            te = (nc.vector if qb == nqb - 1 else tri_eng)
                            te.tensor_mul(
                                pb[:, h, ktile * j:ktile * (j + 1)],
                                pb[:, h, ktile * j:ktile * (j + 1)],
                                tri_sb)
                    mm2_q.append((acc, pb, (t0, t1), qb, tlast))
                    if len(mm2_q) > mm2_defer:
                        flush_mm2()
            while mm2_q:
                flush_mm2()

    nc.compile()
    return nc


def _get_nc(key="main", **kw):
    if key not in _NC_CACHE:
        _NC_CACHE[key] = build(**kw)
    return _NC_CACHE[key]


def _prep_core_inputs(q, k, v, attn_mask, b, n=N, d=D, ktile=KTILE,
                      qblk=QBLK):
    nt = n // ktile
    nqb = n // qblk
    npr = nt // 2
    kT = k[b].T.astype(np.float16)    # [d, n]
    qT = q[b].T.astype(np.float16)
    kpair = np.ascontiguousarray(
        kT.reshape(d, npr, 2, ktile).transpose(2, 0, 1, 3)
    ).reshape(2 * d, npr, ktile)
    qq = np.empty((2 * d, nqb, qblk), dtype=np.float16)
    qq[0:d] = qT.reshape(d, nqb, qblk)
    qq[d:2 * d] = qq[0:d]
    v_aug = np.ones((n, d + 1), dtype=np.float32)
    v_aug[:, :d] = v[b]
    v_aug *= (attn_mask[b] != 0).astype(np.float32)[:, None]
    v_aug = np.ascontiguousarray(
        v_aug.reshape(nt, ktile, d + 1).transpose(1, 0, 2)
    ).astype(np.float16)
    tri = (np.arange(ktile)[None, :] >= np.arange(ktile)[:, None]
           ).astype(np.float16)
    return {"kpair": kpair, "qq": qq, "v_aug": v_aug, "tri": tri}


def kernel(q, k, v, attn_mask):
    global LAST_RESULTS
    q = np.asarray(q, dtype=np.float32)
    k = np.asarray(k, dtype=np.float32)
    v = np.asarray(v, dtype=np.float32)
    attn_mask = np.asarray(attn_mask)

    from concourse.bass_utils import run_bass_kernel_spmd

    nc = _get_nc()
    in_maps = [_prep_core_inputs(q, k, v, attn_mask, b) for b in range(B)]
    trace = bool(os.environ.get("BASS_TRACE"))
    last_err = None
    for attempt in range(3):
        try:
            LAST_RESULTS = run_bass_kernel_spmd(
                nc, in_maps, core_ids=list(range(B)), trace=trace)
            break
        except Exception as e:  # transient device-unrecoverable states clear
            last_err = e
            retryable = any(s in str(e) for s in
                            ("UNAVAILABLE", "unrecoverable", "INTERNAL"))
            if not retryable:
                raise
            import time as _time

            _time.sleep(2.0)
    else:
        raise last_err

    out = np.empty((B, N, D), dtype=np.float32)
    for b in range(B):
        r = np.asarray(LAST_RESULTS.results[b]["outT2"])
        m = r[:, 0, :] + r[:, 1, :]        # merge the K row-strip halves
        out[b] = (m[0:D] / m[D:D + 1]).T   # softmax divide + transpose
    return out
